# revision 21
# baseline (speedup 1.0000x reference)
"""Trainium2 Bass kernel for the 1x1-conv attention block + groupnorm-swish.

Reference computation (B=2, C=128, spatial 16^3 -> N=4096):
    q = wq@query + bq; k = wk@key + bk; v = wv@value + bv   (per batch, [C, N])
    S[i, j] = sum_c q[c,i] k[c,j]; P = softmax_j(S)
    h[c, i] = sum_j v[c,j] P[i,j]
    x = wo@h + bo + value
    out = silu(group_norm(x) * gamma + beta)   (G=32 groups of 4 channels)

Sharding: 8 cores = 2 batches x 4 query-token chunks of 1024 (sequence
parallel). Each core computes the k/v projections for its full batch
(replicated within the batch's 4-core group), its own S^T/softmax/PV chunk,
and group-norm partial sums; one tiny AllReduce produces full-batch group
statistics.

Key layout choices:
- Scores are computed TRANSPOSED (S^T[j, i] = k_tile^T @ q) so the PV
  contraction over j needs no transposes of the softmax output.
- Softmax max-subtraction is skipped (logits ~N(0, 14); exp stays in fp32
  range). The denominator sum_j exp[j, i] is accumulated half on the TENSOR
  ENGINE (an all-ones stationary matrix PSUM-accumulated per exp tile, which
  lands already broadcast across partitions) and half on the VECTOR engine
  (a running tensor_add chain), balancing the two engines.
- The k-projection bias is dropped entirely: it adds a per-query-column
  constant to the logits, which softmax over keys cancels exactly.
- The denominator is applied AFTER the output projection (column scaling
  commutes with channel mixing); the v bias folds into bo_eff = wo@bv + bo on
  the host.
- v^T is produced directly as matmul(value_tile, wv^T) in bf16 (no separate
  v projection, no transposes), four tiles per PSUM bank, batch-cast to SBUF.
- The main loop is software-pipelined: S^T(t+1) is issued before PV(t) so the
  PE never waits on the exp; q/k/v/weights stream in bf16 (halved upload and
  HBM traffic; the matmul accumulation stays fp32).
"""

import sys
import types

import ml_dtypes
import numpy as np

# The axon NTFF-profile hook module is absent from this image's antenv
# package; concourse imports it unconditionally when tracing. Install a
# functional shim (used by the test harness; harmless otherwise).
try:
    import antenv.axon_hooks  # noqa: F401
except ImportError:
    import antenv

    _mod = types.ModuleType("antenv.axon_hooks")
    _hook_box = [None]
    _mod.set_axon_ntff_profile_hook = lambda h: _hook_box.__setitem__(0, h)
    _mod.get_axon_ntff_profile_hook = lambda: _hook_box[0]
    sys.modules["antenv.axon_hooks"] = _mod
    antenv.axon_hooks = _mod
    try:
        from trn_agent_boot.trn_boot import _ntff_profile_via_ctypes

        _mod.set_axon_ntff_profile_hook(
            _ntff_profile_via_ctypes("/opt/axon/libaxon_pjrt.so")
        )
    except Exception:
        pass

import concourse.tile as tile
from concourse import bacc, mybir
from concourse.bass_utils import run_bass_kernel_spmd

B = 2
C = 128
N = 4096
NCORES = 8
CHUNKS = 4  # query-token chunks per batch
NC = N // CHUNKS  # 1024 tokens per core
JT = N // 128  # 32 key tiles of 128
G = 32  # groupnorm groups
EPS = 1e-5
GROUP_ELEMS = float((C // G) * N)  # 16384

R = mybir.dt.float32r
F32 = mybir.dt.float32
BF16 = mybir.dt.bfloat16
AF = mybir.ActivationFunctionType
ALU = mybir.AluOpType

_NC_CACHE = None


def _build():
    nc = bacc.Bacc("TRN2", target_bir_lowering=False, debug=False, num_devices=NCORES)

    q_in = nc.dram_tensor("q_in", [C, NC], BF16, kind="ExternalInput")
    k_in = nc.dram_tensor("k_in", [C, N], BF16, kind="ExternalInput")
    v_in = nc.dram_tensor("v_in", [C, N], BF16, kind="ExternalInput")
    # packed weights: [wqT | wkT | wvT] bf16, plus woT fp32 and the small
    # per-channel vectors [bq | bk | bo_eff | gamma | beta] fp32.
    wqkv_in = nc.dram_tensor("wqkv", [C, 3 * C], BF16, kind="ExternalInput")
    woT_in = nc.dram_tensor("woT", [C, C], R, kind="ExternalInput")
    vecs_in = nc.dram_tensor("vecs", [C, 4], F32, kind="ExternalInput")
    y_out = nc.dram_tensor("y_out", [C, NC], F32, kind="ExternalOutput")

    with tile.TileContext(nc) as tc:
        with (
            tc.tile_pool(name="const", bufs=1) as const,
            tc.tile_pool(name="big", bufs=1) as big,
            tc.tile_pool(name="expp", bufs=4) as expp,
            tc.tile_pool(name="psum", bufs=2, space="PSUM") as psum,
            tc.tile_pool(name="dram", bufs=2, space="DRAM") as dram,
        ):
            # ---- constants / weights (q path first so PE can start early) ----
            wqkv = const.tile([C, 3 * C], BF16)
            woT = const.tile([C, C], R)
            vecs = const.tile([C, 4], F32)
            ones_sb = const.tile([C, C], R)
            e_sb = const.tile([C, G], F32)
            et_sb = const.tile([G, C], F32)
            eps_sb = const.tile([G, 1], F32)

            # PE warm-up: the HAM clock gate needs ~3.4us of sustained PE
            # activity to lift the 1.2GHz cold throttle; spin dummy matmuls
            # on a memset tile while the input DMAs stream in.
            warm_in = const.tile([C, 512], BF16)
            nc.gpsimd.memset(warm_in[:].bitcast(mybir.dt.uint16), 0)
            warm_ps = psum.tile([C, 512], F32, tag="b1", name="warm_ps")
            for _ in range(16):
                nc.tensor.matmul(
                    warm_ps[:], warm_in[:, 0:C], warm_in[:], start=True, stop=True
                )

            nc.sync.dma_start(wqkv[:], wqkv_in[:])
            q_raw = big.tile([C, NC], BF16)
            nc.sync.dma_start(q_raw[:], q_in[:])
            nc.sync.dma_start(vecs[:], vecs_in[:])
            wqT = wqkv[:, 0:C]
            wkT = wqkv[:, C : 2 * C]
            wvT = wqkv[:, 2 * C : 3 * C]
            bq_sb = vecs[:, 0:1]
            boe_sb = vecs[:, 1:2]
            gamma_sb = vecs[:, 2:3]
            beta_sb = vecs[:, 3:4]

            # on-chip constants: all-ones (denominator), group collapse E
            # [C, G] and expand E^T [G, C] one-hot matrices
            nc.gpsimd.memset(ones_sb[:].bitcast(F32), 1.0)
            nc.gpsimd.memset(e_sb[:], 1.0)
            nc.gpsimd.affine_select(
                out=e_sb[:], in_=e_sb[:], compare_op=ALU.is_ge, fill=0.0,
                base=0, pattern=[[-(C // G), G]], channel_multiplier=1,
            )
            nc.gpsimd.affine_select(
                out=e_sb[:], in_=e_sb[:], compare_op=ALU.is_ge, fill=0.0,
                base=C // G - 1, pattern=[[C // G, G]], channel_multiplier=-1,
            )
            nc.gpsimd.memset(et_sb[:], 1.0)
            nc.gpsimd.affine_select(
                out=et_sb[:], in_=et_sb[:], compare_op=ALU.is_ge, fill=0.0,
                base=0, pattern=[[1, C]], channel_multiplier=-(C // G),
            )
            nc.gpsimd.affine_select(
                out=et_sb[:], in_=et_sb[:], compare_op=ALU.is_ge, fill=0.0,
                base=C // G - 1, pattern=[[-1, C]], channel_multiplier=C // G,
            )

            # ---- q projection: q_sb = wq @ query_chunk + bq ----
            q_sb = big.tile([C, NC], R)
            qp = psum.tile([C, NC], F32, tag="st")
            for h in range(NC // 512):
                sl = slice(h * 512, (h + 1) * 512)
                nc.tensor.matmul(qp[:, sl], wqT, q_raw[:, sl], start=True, stop=True)
            nc.vector.tensor_scalar(
                out=q_sb[:], in0=qp[:],
                scalar1=bq_sb, scalar2=None, op0=ALU.add,
            )

            # ---- k projection + v^T, interleaved per 512-chunk DMA ----
            k_raw = big.tile([C, N], BF16)
            k_sb = big.tile([C, N], R)
            v_raw = big.tile([C, N], BF16)
            v_raw3 = v_raw[:].rearrange("c (t j) -> c t j", j=128)
            vt_sb = big.tile([128, JT, C], R)
            for half in range(2):
                hs = slice(half * (N // 2), (half + 1) * (N // 2))
                nc.sync.dma_start(k_raw[:, hs], k_in[:, hs])
                nc.gpsimd.dma_start(v_raw[:, hs], v_in[:, hs])
            for h in range(N // 1024):
                sl = slice(h * 1024, (h + 1) * 1024)
                kp = psum.tile([C, NC], F32, tag="st", name=f"kp{h}")
                for hh in range(2):
                    ssl = slice(h * 1024 + hh * 512, h * 1024 + (hh + 1) * 512)
                    nc.tensor.matmul(
                        kp[:, hh * 512 : (hh + 1) * 512], wkT, k_raw[:, ssl],
                        start=True, stop=True,
                    )
                nc.scalar.activation(out=k_sb[:, sl], in_=kp[:], func=AF.Copy)
                for half in range(2):
                    vw = psum.tile([128, 512], F32, tag="b1", name=f"vw{h}_{half}")
                    for tt in range(4):
                        t = 8 * h + 4 * half + tt
                        nc.tensor.matmul(
                            vw[:, tt * 128 : (tt + 1) * 128],
                            v_raw3[:, t, :], wvT, start=True, stop=True,
                        )
                    nc.vector.tensor_copy(
                        vt_sb[:, 8 * h + 4 * half : 8 * h + 4 * half + 4, :], vw[:]
                    )

            # remaining late inputs
            nc.sync.dma_start(woT[:], woT_in[:])
            nc.vector.memset(eps_sb[:], EPS)
            warm_sb = const.tile([G, 1], F32)
            nc.scalar.activation(out=warm_sb[:], in_=eps_sb[:], func=AF.Sqrt)
            nc.scalar.activation(out=warm_sb[:], in_=eps_sb[:], func=AF.Silu)
            nc.scalar.activation(out=warm_sb[:], in_=eps_sb[:], func=AF.Exp)
            r_sb = big.tile([C, NC], F32)
            nc.vector.tensor_scalar(
                out=r_sb[:], in0=v_raw[:, 0:NC],
                scalar1=boe_sb, scalar2=None, op0=ALU.add,
            )

            # ---- main attention loop over 32 key tiles ----
            # per tile: S^T = k_tile^T @ q (psum) -> exp (ACT, ->sbuf fp32r)
            #           h  += v^T_tile @ exp     (PSUM accumulate)
            #           db += ones    @ exp      (PSUM accumulate = denominator)
            k_sb3 = k_sb[:].rearrange("c (t j) -> c t j", j=128)
            h_ps = psum.tile([C, NC], F32, tag="h", bufs=1)
            db_ps = psum.tile([C, 512], F32, tag="b1")
            acc_sb = big.tile([128, 512], F32)

            # software-pipelined: the PE computes S^T(t+1) while ACT
            # exponentiates tile t, then immediately consumes exp(t).
            def qk(t, st):
                for h in range(NC // 512):
                    sl = slice(h * 512, (h + 1) * 512)
                    nc.tensor.matmul(
                        st[:, sl], k_sb3[:, t, :], q_sb[:, sl],
                        start=True, stop=True,
                    )

            st_tiles = {}
            st_tiles[0] = psum.tile([128, NC], F32, tag="st", name="st0")
            qk(0, st_tiles[0])
            for t in range(JT):
                if t + 1 < JT:
                    st_tiles[t + 1] = psum.tile([128, NC], F32, tag="st", name=f"st{t + 1}")
                    qk(t + 1, st_tiles[t + 1])
                exp_t = expp.tile([128, NC], R, tag="exp")
                nc.scalar.activation(out=exp_t[:], in_=st_tiles.pop(t)[:], func=AF.Exp)
                for h in range(NC // 512):
                    sl = slice(h * 512, (h + 1) * 512)
                    nc.tensor.matmul(
                        h_ps[:, sl], vt_sb[:, t, :], exp_t[:, sl],
                        start=(t == 0), stop=(t == JT - 1), skip_group_check=True,
                    )
                nc.tensor.matmul(
                    db_ps[:], ones_sb[:], exp_t[:, 0:512],
                    start=(t == 0), stop=(t == JT - 1), skip_group_check=True,
                )
                if t == 0:
                    nc.vector.tensor_copy(acc_sb[:], exp_t[:, 512:NC].bitcast(F32))
                else:
                    nc.vector.tensor_add(
                        acc_sb[:], acc_sb[:], exp_t[:, 512:NC].bitcast(F32)
                    )

            # ---- 1/denominator ----
            db2_ps = psum.tile([C, 512], F32, tag="b1")
            nc.tensor.matmul(db2_ps[:], ones_sb[:].bitcast(F32), acc_sb[:],
                             start=True, stop=True)
            dinv_sb = big.tile([C, NC], F32)
            nc.vector.reciprocal(dinv_sb[:, 0:512], db_ps[:])
            nc.vector.reciprocal(dinv_sb[:, 512:NC], db2_ps[:])

            # ---- output projection; x = o * dinv + (vres + bo_eff) ----
            h_sb = big.tile([C, NC], R)
            nc.scalar.activation(out=h_sb[:], in_=h_ps[:], func=AF.Copy)
            o_ps = psum.tile([C, NC], F32, tag="st")
            for h in range(NC // 512):
                sl = slice(h * 512, (h + 1) * 512)
                nc.tensor.matmul(o_ps[:, sl], woT[:], h_sb[:, sl], start=True, stop=True)
            x_sb = big.tile([C, NC], F32)
            nc.vector.tensor_mul(x_sb[:], o_ps[:], dinv_sb[:])
            nc.vector.tensor_add(x_sb[:], x_sb[:], r_sb[:])

            # ---- groupnorm partial stats: per-channel [mean, E[x^2]] ----
            bstats = big.tile([C, 2, nc.vector.BN_STATS_DIM], F32)
            for hh in range(2):
                nc.vector.bn_stats(
                    out=bstats[:, hh, :], in_=x_sb[:, hh * 512 : (hh + 1) * 512]
                )
            mv = big.tile([C, nc.vector.BN_AGGR_DIM], F32)
            nc.vector.bn_aggr(out=mv[:], in_=bstats[:])
            rowstats = big.tile([C, 2], F32)
            nc.vector.tensor_copy(rowstats[:, 0:1], mv[:, 0:1])
            nc.vector.tensor_mul(rowstats[:, 1:2], mv[:, 0:1], mv[:, 0:1])
            nc.vector.tensor_add(rowstats[:, 1:2], rowstats[:, 1:2], mv[:, 1:2])
            gs_ps = psum.tile([G, 2], F32, tag="b1")
            nc.tensor.matmul(gs_ps[:], e_sb[:], rowstats[:], start=True, stop=True)
            gs_sb = big.tile([G, 2], F32)
            nc.vector.tensor_copy(gs_sb[:], gs_ps[:])

            # ---- AllReduce partial stats within each batch's 4-core group ----
            cc_in = dram.tile([G, 2], F32)
            cc_out = dram.tile([G, 2], F32)
            nc.sync.dma_start(cc_in[:], gs_sb[:])
            nc.gpsimd.collective_compute(
                "AllReduce",
                ALU.add,
                replica_groups=[[0, 1, 2, 3], [4, 5, 6, 7]],
                ins=[cc_in.opt()],
                outs=[cc_out.opt()],
            )
            own = big.tile([G, 2], F32)
            nc.sync.dma_start(own[:], cc_out[:])

            # ---- group mean / rstd -> per-channel scale+bias ----
            msr = big.tile([G, 2], F32)  # [mean, rstd]
            nc.vector.tensor_scalar(
                out=msr[:], in0=own[:], scalar1=1.0 / 16.0, scalar2=None,
                op0=ALU.mult,
            )
            m2 = big.tile([G, 1], F32)
            nc.vector.tensor_mul(m2[:], msr[:, 0:1], msr[:, 0:1])
            var = big.tile([G, 1], F32)
            nc.vector.tensor_sub(var[:], msr[:, 1:2], m2[:])
            sd = big.tile([G, 1], F32)
            nc.scalar.activation(
                out=sd[:], in_=var[:], func=AF.Sqrt, bias=eps_sb[:], scale=1.0
            )
            nc.vector.reciprocal(msr[:, 1:2], sd[:])
            exp_ps = psum.tile([C, 2], F32, tag="b1")
            nc.tensor.matmul(exp_ps[:], et_sb[:], msr[:], start=True, stop=True)
            mr_sb = big.tile([C, 2], F32)
            nc.vector.tensor_copy(mr_sb[:], exp_ps[:])
            fs_sb = big.tile([C, 1], F32)
            nc.vector.tensor_mul(fs_sb[:], mr_sb[:, 1:2], gamma_sb[:])
            fb_sb = big.tile([C, 1], F32)
            nc.vector.tensor_mul(fb_sb[:], mr_sb[:, 0:1], fs_sb[:])
            nc.vector.tensor_sub(fb_sb[:], beta_sb[:], fb_sb[:])

            # ---- out = silu(fs * x + fb) ----
            y_sb = big.tile([C, NC], F32)
            nc.scalar.activation(
                out=y_sb[:], in_=x_sb[:], func=AF.Silu, bias=fb_sb[:], scale=fs_sb[:]
            )
            nc.sync.dma_start(y_out[:], y_sb[:])

    nc.compile()
    return nc


def _get_nc():
    global _NC_CACHE
    if _NC_CACHE is None:
        _NC_CACHE = _build()
    return _NC_CACHE


def _in_maps(query, key, value, wq, bq, wk, bk, wv, bv, wo, bo, gamma, beta):
    f32 = lambda a: np.ascontiguousarray(np.asarray(a, dtype=np.float32))
    q = f32(query).reshape(B, C, N)
    k = f32(key).reshape(B, C, N)
    v = f32(value).reshape(B, C, N)
    wq, wk, wv, wo = f32(wq), f32(wk), f32(wv), f32(wo)
    bo_eff = (wo @ f32(bv).reshape(C) + f32(bo).reshape(C)).astype(np.float32)

    wqkv = np.concatenate([wq.T, wk.T, wv.T], axis=1).astype(ml_dtypes.bfloat16)
    vecs = np.stack(
        [f32(bq).reshape(C), bo_eff,
         f32(gamma).reshape(C), f32(beta).reshape(C)], axis=1
    ).astype(np.float32)
    shared = {
        "wqkv": np.ascontiguousarray(wqkv),
        "woT": np.ascontiguousarray(wo.T),
        "vecs": np.ascontiguousarray(vecs),
    }
    maps = []
    for p in range(NCORES):
        b, ch = divmod(p, CHUNKS)
        sl = slice(ch * NC, (ch + 1) * NC)
        # rotate the key/value token axis so this core's chunk sits at j=0;
        # attention is permutation-invariant over keys, and the residual
        # slice becomes v_in[:, 0:NC] at the same offset on every core.
        rot = np.roll(np.arange(N), -ch * NC)
        maps.append(
            {
                "q_in": np.ascontiguousarray(q[b][:, sl]).astype(ml_dtypes.bfloat16),
                "k_in": np.ascontiguousarray(k[b][:, rot]).astype(ml_dtypes.bfloat16),
                "v_in": np.ascontiguousarray(v[b][:, rot]).astype(ml_dtypes.bfloat16),
                **shared,
            }
        )
    return maps


def kernel(query, key, value, wq, bq, wk, bk, wv, bv, wo, bo, gamma, beta):
    nc = _get_nc()
    maps = _in_maps(query, key, value, wq, bq, wk, bk, wv, bv, wo, bo, gamma, beta)
    res = run_bass_kernel_spmd(nc, maps, list(range(NCORES)))
    out = np.empty((B, C, N), dtype=np.float32)
    for p in range(NCORES):
        b, ch = divmod(p, CHUNKS)
        out[b][:, ch * NC : (ch + 1) * NC] = res.results[p]["y_out"]
    return out.reshape(B, C, 16, 16, 16)


# revision 22
# speedup vs baseline: 1.0743x; 1.0743x over previous
"""Trainium2 Bass kernel for the 1x1-conv attention block + groupnorm-swish.

Reference computation (B=2, C=128, spatial 16^3 -> N=4096):
    q = wq@query + bq; k = wk@key + bk; v = wv@value + bv   (per batch, [C, N])
    S[i, j] = sum_c q[c,i] k[c,j]; P = softmax_j(S)
    h[c, i] = sum_j v[c,j] P[i,j]
    x = wo@h + bo + value
    out = silu(group_norm(x) * gamma + beta)   (G=32 groups of 4 channels)

Sharding: 8 cores = 2 batches x 4 query-token chunks of 1024 (sequence
parallel). Each core computes the k/v projections for its full batch
(replicated within the batch's 4-core group), its own S^T/softmax/PV chunk,
and group-norm partial sums; one tiny AllReduce produces full-batch group
statistics.

Key layout choices:
- Scores are computed TRANSPOSED (S^T[j, i] = k_tile^T @ q) so the PV
  contraction over j needs no transposes of the softmax output.
- Softmax max-subtraction is skipped (logits ~N(0, 14); exp stays in fp32
  range). The denominator sum_j exp[j, i] is accumulated half on the TENSOR
  ENGINE (an all-ones stationary matrix PSUM-accumulated per exp tile, which
  lands already broadcast across partitions) and half on the VECTOR engine
  (a running tensor_add chain), balancing the two engines.
- The k-projection bias is dropped entirely: it adds a per-query-column
  constant to the logits, which softmax over keys cancels exactly.
- The denominator is applied AFTER the output projection (column scaling
  commutes with channel mixing); the v bias folds into bo_eff = wo@bv + bo on
  the host.
- v^T is produced directly as matmul(value_tile, wv^T) in bf16 (no separate
  v projection, no transposes), four tiles per PSUM bank, batch-cast to SBUF.
- The main loop is software-pipelined: S^T(t+1) is issued before PV(t) so the
  PE never waits on the exp; q/k/v/weights stream in bf16 (halved upload and
  HBM traffic; the matmul accumulation stays fp32).
"""

import sys
import types

import ml_dtypes
import numpy as np

# The axon NTFF-profile hook module is absent from this image's antenv
# package; concourse imports it unconditionally when tracing. Install a
# functional shim (used by the test harness; harmless otherwise).
try:
    import antenv.axon_hooks  # noqa: F401
except ImportError:
    import antenv

    _mod = types.ModuleType("antenv.axon_hooks")
    _hook_box = [None]
    _mod.set_axon_ntff_profile_hook = lambda h: _hook_box.__setitem__(0, h)
    _mod.get_axon_ntff_profile_hook = lambda: _hook_box[0]
    sys.modules["antenv.axon_hooks"] = _mod
    antenv.axon_hooks = _mod
    try:
        from trn_agent_boot.trn_boot import _ntff_profile_via_ctypes

        _mod.set_axon_ntff_profile_hook(
            _ntff_profile_via_ctypes("/opt/axon/libaxon_pjrt.so")
        )
    except Exception:
        pass

import concourse.tile as tile
from concourse import bacc, mybir
from concourse.bass_utils import run_bass_kernel_spmd

B = 2
C = 128
N = 4096
NCORES = 8
CHUNKS = 4  # query-token chunks per batch
NC = N // CHUNKS  # 1024 tokens per core
JT = N // 128  # 32 key tiles of 128
G = 32  # groupnorm groups
EPS = 1e-5
GROUP_ELEMS = float((C // G) * N)  # 16384

R = mybir.dt.float32r
F32 = mybir.dt.float32
BF16 = mybir.dt.bfloat16
AF = mybir.ActivationFunctionType
ALU = mybir.AluOpType

_NC_CACHE = None


def _build():
    nc = bacc.Bacc("TRN2", target_bir_lowering=False, debug=False, num_devices=NCORES)

    q_in = nc.dram_tensor("q_in", [C, NC], BF16, kind="ExternalInput")
    k_in = nc.dram_tensor("k_in", [C, N], BF16, kind="ExternalInput")
    v_in = nc.dram_tensor("v_in", [C, N], BF16, kind="ExternalInput")
    # packed weights: [wqT | wkT | wvT] bf16, plus woT fp32 and the small
    # per-channel vectors [bq | bk | bo_eff | gamma | beta] fp32.
    wqkv_in = nc.dram_tensor("wqkv", [C, 3 * C], BF16, kind="ExternalInput")
    woT_in = nc.dram_tensor("woT", [C, C], R, kind="ExternalInput")
    vecs_in = nc.dram_tensor("vecs", [C, 4], F32, kind="ExternalInput")
    y_out = nc.dram_tensor("y_out", [C, NC], F32, kind="ExternalOutput")

    with tile.TileContext(nc) as tc:
        with (
            tc.tile_pool(name="const", bufs=1) as const,
            tc.tile_pool(name="big", bufs=1) as big,
            tc.tile_pool(name="expp", bufs=4) as expp,
            tc.tile_pool(name="psum", bufs=2, space="PSUM") as psum,
            tc.tile_pool(name="dram", bufs=2, space="DRAM") as dram,
        ):
            # ---- constants / weights (q path first so PE can start early) ----
            wqkv = const.tile([C, 3 * C], BF16)
            woT = const.tile([C, C], R)
            vecs = const.tile([C, 4], F32)
            ones_sb = const.tile([C, C], R)
            e_sb = const.tile([C, G], F32)
            et_sb = const.tile([G, C], F32)
            eps_sb = const.tile([G, 1], F32)

            # PE warm-up: the HAM clock gate needs ~3.4us of sustained PE
            # activity to lift the 1.2GHz cold throttle; spin dummy matmuls
            # on a memset tile while the input DMAs stream in.
            warm_in = const.tile([C, 512], BF16)
            nc.gpsimd.memset(warm_in[:].bitcast(mybir.dt.uint16), 0)
            warm_ps = psum.tile([C, 512], F32, tag="b1", name="warm_ps")
            for _ in range(24):
                nc.tensor.matmul(
                    warm_ps[:], warm_in[:, 0:C], warm_in[:], start=True, stop=True
                )

            nc.sync.dma_start(wqkv[:], wqkv_in[:])
            q_raw = big.tile([C, NC], BF16)
            nc.sync.dma_start(q_raw[:], q_in[:])
            nc.sync.dma_start(vecs[:], vecs_in[:])
            wqT = wqkv[:, 0:C]
            wkT = wqkv[:, C : 2 * C]
            wvT = wqkv[:, 2 * C : 3 * C]
            bq_sb = vecs[:, 0:1]
            boe_sb = vecs[:, 1:2]
            gamma_sb = vecs[:, 2:3]
            beta_sb = vecs[:, 3:4]

            # on-chip constants: all-ones (denominator), group collapse E
            # [C, G] and expand E^T [G, C] one-hot matrices
            nc.gpsimd.memset(ones_sb[:].bitcast(F32), 1.0)
            nc.gpsimd.memset(e_sb[:], 1.0)
            nc.gpsimd.affine_select(
                out=e_sb[:], in_=e_sb[:], compare_op=ALU.is_ge, fill=0.0,
                base=0, pattern=[[-(C // G), G]], channel_multiplier=1,
            )
            nc.gpsimd.affine_select(
                out=e_sb[:], in_=e_sb[:], compare_op=ALU.is_ge, fill=0.0,
                base=C // G - 1, pattern=[[C // G, G]], channel_multiplier=-1,
            )
            nc.gpsimd.memset(et_sb[:], 1.0)
            nc.gpsimd.affine_select(
                out=et_sb[:], in_=et_sb[:], compare_op=ALU.is_ge, fill=0.0,
                base=0, pattern=[[1, C]], channel_multiplier=-(C // G),
            )
            nc.gpsimd.affine_select(
                out=et_sb[:], in_=et_sb[:], compare_op=ALU.is_ge, fill=0.0,
                base=C // G - 1, pattern=[[-1, C]], channel_multiplier=C // G,
            )

            # ---- q projection: q_sb = wq @ query_chunk + bq ----
            q_sb = big.tile([C, NC], R)
            qp = psum.tile([C, NC], F32, tag="st")
            for h in range(NC // 512):
                sl = slice(h * 512, (h + 1) * 512)
                nc.tensor.matmul(qp[:, sl], wqT, q_raw[:, sl], start=True, stop=True)
            nc.vector.tensor_scalar(
                out=q_sb[:], in0=qp[:],
                scalar1=bq_sb, scalar2=None, op0=ALU.add,
            )

            # ---- k projection + v^T, interleaved per 512-chunk DMA ----
            k_raw = big.tile([C, N], BF16)
            k_sb = big.tile([C, N], R)
            v_raw = big.tile([C, N], BF16)
            v_raw3 = v_raw[:].rearrange("c (t j) -> c t j", j=128)
            vt_sb = big.tile([128, JT, C], R)
            for qtr in range(4):
                qs = slice(qtr * (N // 4), (qtr + 1) * (N // 4))
                nc.sync.dma_start(k_raw[:, qs], k_in[:, qs])
                nc.gpsimd.dma_start(v_raw[:, qs], v_in[:, qs])
            for h in range(N // 1024):
                sl = slice(h * 1024, (h + 1) * 1024)
                kp = psum.tile([C, NC], F32, tag="st", name=f"kp{h}")
                for hh in range(2):
                    ssl = slice(h * 1024 + hh * 512, h * 1024 + (hh + 1) * 512)
                    nc.tensor.matmul(
                        kp[:, hh * 512 : (hh + 1) * 512], wkT, k_raw[:, ssl],
                        start=True, stop=True,
                    )
                nc.scalar.activation(out=k_sb[:, sl], in_=kp[:], func=AF.Copy)
                for half in range(2):
                    vw = psum.tile([128, 512], F32, tag="b1", name=f"vw{h}_{half}")
                    for tt in range(4):
                        t = 8 * h + 4 * half + tt
                        nc.tensor.matmul(
                            vw[:, tt * 128 : (tt + 1) * 128],
                            v_raw3[:, t, :], wvT, start=True, stop=True,
                        )
                    nc.vector.tensor_copy(
                        vt_sb[:, 8 * h + 4 * half : 8 * h + 4 * half + 4, :], vw[:]
                    )

            # remaining late inputs
            nc.sync.dma_start(woT[:], woT_in[:])
            nc.vector.memset(eps_sb[:], EPS)
            warm_sb = const.tile([G, 1], F32)
            nc.scalar.activation(out=warm_sb[:], in_=eps_sb[:], func=AF.Sqrt)
            nc.scalar.activation(out=warm_sb[:], in_=eps_sb[:], func=AF.Silu)
            nc.scalar.activation(out=warm_sb[:], in_=eps_sb[:], func=AF.Exp)
            r_sb = big.tile([C, NC], F32)
            nc.vector.tensor_scalar(
                out=r_sb[:], in0=v_raw[:, 0:NC],
                scalar1=boe_sb, scalar2=None, op0=ALU.add,
            )

            # ---- main attention loop over 32 key tiles ----
            # per tile: S^T = k_tile^T @ q (psum) -> exp (ACT, ->sbuf fp32r)
            #           h  += v^T_tile @ exp     (PSUM accumulate)
            #           db += ones    @ exp      (PSUM accumulate = denominator)
            k_sb3 = k_sb[:].rearrange("c (t j) -> c t j", j=128)
            h_ps = psum.tile([C, NC], F32, tag="h", bufs=1)
            db_ps = psum.tile([C, 512], F32, tag="b1")
            acc_sb = big.tile([128, 512], F32)

            # software-pipelined: the PE computes S^T(t+1) while ACT
            # exponentiates tile t, then immediately consumes exp(t).
            def qk(t, st):
                for h in range(NC // 512):
                    sl = slice(h * 512, (h + 1) * 512)
                    nc.tensor.matmul(
                        st[:, sl], k_sb3[:, t, :], q_sb[:, sl],
                        start=True, stop=True,
                    )

            st_tiles = {}
            st_tiles[0] = psum.tile([128, NC], F32, tag="st", name="st0")
            qk(0, st_tiles[0])
            for t in range(JT):
                if t + 1 < JT:
                    st_tiles[t + 1] = psum.tile([128, NC], F32, tag="st", name=f"st{t + 1}")
                    qk(t + 1, st_tiles[t + 1])
                exp_t = expp.tile([128, NC], R, tag="exp")
                nc.scalar.activation(out=exp_t[:], in_=st_tiles.pop(t)[:], func=AF.Exp)
                for h in range(NC // 512):
                    sl = slice(h * 512, (h + 1) * 512)
                    nc.tensor.matmul(
                        h_ps[:, sl], vt_sb[:, t, :], exp_t[:, sl],
                        start=(t == 0), stop=(t == JT - 1), skip_group_check=True,
                    )
                nc.tensor.matmul(
                    db_ps[:], ones_sb[:], exp_t[:, 0:512],
                    start=(t == 0), stop=(t == JT - 1), skip_group_check=True,
                )
                if t == 0:
                    nc.vector.tensor_copy(acc_sb[:], exp_t[:, 512:NC].bitcast(F32))
                else:
                    nc.vector.tensor_add(
                        acc_sb[:], acc_sb[:], exp_t[:, 512:NC].bitcast(F32)
                    )

            # ---- 1/denominator ----
            db2_ps = psum.tile([C, 512], F32, tag="b1")
            nc.tensor.matmul(db2_ps[:], ones_sb[:].bitcast(F32), acc_sb[:],
                             start=True, stop=True)
            dinv_sb = big.tile([C, NC], F32)
            nc.vector.reciprocal(dinv_sb[:, 0:512], db_ps[:])
            nc.vector.reciprocal(dinv_sb[:, 512:NC], db2_ps[:])

            # ---- output projection; x = o * dinv + (vres + bo_eff) ----
            h_sb = big.tile([C, NC], R)
            nc.scalar.activation(out=h_sb[:], in_=h_ps[:], func=AF.Copy)
            o_ps = psum.tile([C, NC], F32, tag="st")
            for h in range(NC // 512):
                sl = slice(h * 512, (h + 1) * 512)
                nc.tensor.matmul(o_ps[:, sl], woT[:], h_sb[:, sl], start=True, stop=True)
            x_sb = big.tile([C, NC], F32)
            nc.vector.tensor_mul(x_sb[:], o_ps[:], dinv_sb[:])
            nc.vector.tensor_add(x_sb[:], x_sb[:], r_sb[:])

            # ---- groupnorm partial stats: per-channel [mean, E[x^2]] ----
            bstats = big.tile([C, 2, nc.vector.BN_STATS_DIM], F32)
            for hh in range(2):
                nc.vector.bn_stats(
                    out=bstats[:, hh, :], in_=x_sb[:, hh * 512 : (hh + 1) * 512]
                )
            mv = big.tile([C, nc.vector.BN_AGGR_DIM], F32)
            nc.vector.bn_aggr(out=mv[:], in_=bstats[:])
            rowstats = big.tile([C, 2], F32)
            nc.vector.tensor_copy(rowstats[:, 0:1], mv[:, 0:1])
            nc.vector.tensor_mul(rowstats[:, 1:2], mv[:, 0:1], mv[:, 0:1])
            nc.vector.tensor_add(rowstats[:, 1:2], rowstats[:, 1:2], mv[:, 1:2])
            gs_ps = psum.tile([G, 2], F32, tag="b1")
            nc.tensor.matmul(gs_ps[:], e_sb[:], rowstats[:], start=True, stop=True)
            gs_sb = big.tile([G, 2], F32)
            nc.vector.tensor_copy(gs_sb[:], gs_ps[:])

            # ---- AllReduce partial stats within each batch's 4-core group ----
            cc_in = dram.tile([G, 2], F32)
            cc_out = dram.tile([G, 2], F32)
            nc.sync.dma_start(cc_in[:], gs_sb[:])
            nc.gpsimd.collective_compute(
                "AllReduce",
                ALU.add,
                replica_groups=[[0, 1, 2, 3], [4, 5, 6, 7]],
                ins=[cc_in.opt()],
                outs=[cc_out.opt()],
            )
            own = big.tile([G, 2], F32)
            nc.sync.dma_start(own[:], cc_out[:])

            # ---- group mean / rstd -> per-channel scale+bias ----
            msr = big.tile([G, 2], F32)  # [mean, rstd]
            nc.vector.tensor_scalar(
                out=msr[:], in0=own[:], scalar1=1.0 / 16.0, scalar2=None,
                op0=ALU.mult,
            )
            m2 = big.tile([G, 1], F32)
            nc.vector.tensor_mul(m2[:], msr[:, 0:1], msr[:, 0:1])
            var = big.tile([G, 1], F32)
            nc.vector.tensor_sub(var[:], msr[:, 1:2], m2[:])
            sd = big.tile([G, 1], F32)
            nc.scalar.activation(
                out=sd[:], in_=var[:], func=AF.Sqrt, bias=eps_sb[:], scale=1.0
            )
            nc.vector.reciprocal(msr[:, 1:2], sd[:])
            exp_ps = psum.tile([C, 2], F32, tag="b1")
            nc.tensor.matmul(exp_ps[:], et_sb[:], msr[:], start=True, stop=True)
            mr_sb = big.tile([C, 2], F32)
            nc.vector.tensor_copy(mr_sb[:], exp_ps[:])
            fs_sb = big.tile([C, 1], F32)
            nc.vector.tensor_mul(fs_sb[:], mr_sb[:, 1:2], gamma_sb[:])
            fb_sb = big.tile([C, 1], F32)
            nc.vector.tensor_mul(fb_sb[:], mr_sb[:, 0:1], fs_sb[:])
            nc.vector.tensor_sub(fb_sb[:], beta_sb[:], fb_sb[:])

            # ---- out = silu(fs * x + fb) ----
            y_sb = big.tile([C, NC], F32)
            nc.scalar.activation(
                out=y_sb[:], in_=x_sb[:], func=AF.Silu, bias=fb_sb[:], scale=fs_sb[:]
            )
            nc.sync.dma_start(y_out[:], y_sb[:])

    nc.compile()
    return nc


def _get_nc():
    global _NC_CACHE
    if _NC_CACHE is None:
        _NC_CACHE = _build()
    return _NC_CACHE


def _in_maps(query, key, value, wq, bq, wk, bk, wv, bv, wo, bo, gamma, beta):
    f32 = lambda a: np.ascontiguousarray(np.asarray(a, dtype=np.float32))
    q = f32(query).reshape(B, C, N)
    k = f32(key).reshape(B, C, N)
    v = f32(value).reshape(B, C, N)
    wq, wk, wv, wo = f32(wq), f32(wk), f32(wv), f32(wo)
    bo_eff = (wo @ f32(bv).reshape(C) + f32(bo).reshape(C)).astype(np.float32)

    wqkv = np.concatenate([wq.T, wk.T, wv.T], axis=1).astype(ml_dtypes.bfloat16)
    vecs = np.stack(
        [f32(bq).reshape(C), bo_eff,
         f32(gamma).reshape(C), f32(beta).reshape(C)], axis=1
    ).astype(np.float32)
    shared = {
        "wqkv": np.ascontiguousarray(wqkv),
        "woT": np.ascontiguousarray(wo.T),
        "vecs": np.ascontiguousarray(vecs),
    }
    maps = []
    for p in range(NCORES):
        b, ch = divmod(p, CHUNKS)
        sl = slice(ch * NC, (ch + 1) * NC)
        # rotate the key/value token axis so this core's chunk sits at j=0;
        # attention is permutation-invariant over keys, and the residual
        # slice becomes v_in[:, 0:NC] at the same offset on every core.
        rot = np.roll(np.arange(N), -ch * NC)
        maps.append(
            {
                "q_in": np.ascontiguousarray(q[b][:, sl]).astype(ml_dtypes.bfloat16),
                "k_in": np.ascontiguousarray(k[b][:, rot]).astype(ml_dtypes.bfloat16),
                "v_in": np.ascontiguousarray(v[b][:, rot]).astype(ml_dtypes.bfloat16),
                **shared,
            }
        )
    return maps


def kernel(query, key, value, wq, bq, wk, bk, wv, bv, wo, bo, gamma, beta):
    nc = _get_nc()
    maps = _in_maps(query, key, value, wq, bq, wk, bk, wv, bv, wo, bo, gamma, beta)
    res = run_bass_kernel_spmd(nc, maps, list(range(NCORES)))
    out = np.empty((B, C, N), dtype=np.float32)
    for p in range(NCORES):
        b, ch = divmod(p, CHUNKS)
        out[b][:, ch * NC : (ch + 1) * NC] = res.results[p]["y_out"]
    return out.reshape(B, C, 16, 16, 16)
